# revision 2
# baseline (speedup 1.0000x reference)
"""Trainium2 Bass kernel for nn_EnhancedDistillationLoss (bf16 streaming).

Distillation loss = CE_W * masked-CE(student_logits, labels)
                  + KL_W * masked-KL(uniform-teacher || student @ TEMP)

Strategy (data parallel over the 8 NeuronCores):
  - Cast logits to bf16 on the host (loss rel-err ~2e-5, measured): halves
    HBM traffic; the 8-core-contended DMA floor measured ~116 us/core
    (38.8 MB/core at ~333 GB/s/core).
  - Flatten to [B*S, V] rows; core c owns rows [128c, 128c+128) = 128 SBUF
    partitions, vocab streamed in TILE_W tiles.
  - Per tile, three reductions while x is in SBUF. HW-measured op rates:
    ACT is 1 elem/cyc @1.2 GHz; ANY DVE op with accum_out runs 1x
    (~1.04 cyc/elem @0.96 GHz); plain tensor_tensor bf16 runs 2x (0.47).
      ACT : y = exp(x/2) bf16, accum_out -> S2 += sum(exp(x/2))
      S1 += sum(y*y): split between ACT (Square+accum, BACT of tiles) and
            DVE (scalar_tensor_tensor y*y with accum, 1x) to balance engines
      T += sum(x): pairwise TT-add folds (2x rate) shrink x by 2^T_FOLDS,
            then a short 1x tensor_scalar accum tail per tile
  - x[r, label_r]: one GPSIMD indirect DMA gathers the 256B-aligned 128-elem
    bf16 block containing each label, then a one-hot dot selects it.
  - Host combines per-row sums exactly like the reference (float64):
      ce  = mean_valid(log S1 - x[label])
      slp_sum = T/2 - V*log S2
      kl  = mean_mask(V*p*log p - p*slp_sum) * TEMP^2
"""

import functools
import os
from contextlib import ExitStack

import numpy as np
import ml_dtypes

import concourse.bacc as bacc
import concourse.tile as tile
from concourse import bass, mybir
from concourse.bass_utils import run_bass_kernel_spmd

B, S, V = 2, 512, 151643
TEMP = 2.0
CE_W, KL_W = 1.0, 0.5
N_CORES = 8
P = 128  # rows per core == SBUF partitions
TILE_W = 8192  # vocab tile width (bf16: 16KB/partition, 2MB per DMA)
X_BUFS = 6
Y_BUFS = 3
# Fraction of tiles whose sum-of-squares runs on ACT (Square+accum) instead
# of DVE (scalar_tensor_tensor, 1x): tile t -> ACT iff (t*BN) % BD < BN.
BACT_NUM, BACT_DEN = 5, 12
T_FOLDS = 4  # pairwise TT-add folds of x before the 1x accum tail

f32 = mybir.dt.float32
bf16 = mybir.dt.bfloat16
i32 = mybir.dt.int32

GATHER_BLK = 128  # indirect-DMA gather granularity (128 bf16 = 256 B)


def _ceil_div(a, b):
    return -(-a // b)


class TileContextWrapper:
    """TileContext + ExitStack in one `with`."""

    def __init__(self, nc):
        self.nc = nc

    def __enter__(self):
        self.ctx = ExitStack()
        self.ctx.__enter__()
        self.tc = tile.TileContext(self.nc)
        self.tc.__enter__()
        return self.tc, self.ctx

    def __exit__(self, *exc):
        self.ctx.__exit__(*exc)
        return self.tc.__exit__(*exc)


def build_kernel(v=V, tile_w=TILE_W, p=P):
    nc = bacc.Bacc("TRN2", target_bir_lowering=False, debug=False)
    x = nc.dram_tensor("x", [p, v], bf16, kind="ExternalInput")
    gidx = nc.dram_tensor("gidx", [p, 1], i32, kind="ExternalInput")
    onehot = nc.dram_tensor("onehot", [p, GATHER_BLK], bf16, kind="ExternalInput")
    stats = nc.dram_tensor("stats", [p, 4], f32, kind="ExternalOutput")

    n_tiles = _ceil_div(v, tile_w)

    with TileContextWrapper(nc) as (tc, ctx):
        xp = ctx.enter_context(tc.tile_pool(name="xp", bufs=X_BUFS))
        yp = ctx.enter_context(tc.tile_pool(name="yp", bufs=Y_BUFS))
        accp = ctx.enter_context(tc.tile_pool(name="accp", bufs=1))

        s1p = accp.tile([p, n_tiles], f32)
        s2p = accp.tile([p, n_tiles], f32)
        txp = accp.tile([p, n_tiles], f32)
        dve_scr = accp.tile([p, tile_w], bf16)
        act_dummy = accp.tile([p, 1], bf16)
        idx_sb = accp.tile([p, 1], i32)
        oh_sb = accp.tile([p, GATHER_BLK], bf16)
        blk_sb = accp.tile([p, GATHER_BLK], bf16)
        blk_dummy = accp.tile([p, 1], f32)
        stats_sb = accp.tile([p, 4], f32)

        # gather: stats col 3 <- x[r, label_r] via a 256B-aligned block
        # indirect DMA + one-hot dot (single-element indirect DMA faults).
        nc.sync.dma_start(out=idx_sb[:], in_=gidx[:])
        nc.sync.dma_start(out=oh_sb[:], in_=onehot[:])
        nc.gpsimd.indirect_dma_start(
            out=blk_sb[:],
            out_offset=None,
            in_=x[:]
            .rearrange("p v -> (p v)")
            .rearrange("(a b) -> a b", b=GATHER_BLK),
            in_offset=bass.IndirectOffsetOnAxis(ap=idx_sb[:, :1], axis=0),
        )
        nc.vector.scalar_tensor_tensor(
            out=blk_dummy[:].broadcast_to((p, GATHER_BLK)),
            in0=blk_sb[:],
            scalar=1.0,
            in1=oh_sb[:],
            op0=mybir.AluOpType.mult,
            op1=mybir.AluOpType.mult,
            accum_out=stats_sb[:, 3:4],
        )

        for t in range(n_tiles):
            w0 = t * tile_w
            wt = min(tile_w, v - w0)
            xt = xp.tile([p, tile_w], bf16, tag="x")
            yt = yp.tile([p, tile_w], bf16, tag="y")
            nc.sync.dma_start(out=xt[:, :wt], in_=x[:, w0 : w0 + wt])
            nc.scalar.activation(
                out=yt[:, :wt],
                in_=xt[:, :wt],
                func=mybir.ActivationFunctionType.Exp,
                scale=0.5,
                accum_out=s2p[:, t : t + 1],
            )
            if (t * BACT_NUM) % BACT_DEN < BACT_NUM:
                nc.scalar.activation(
                    out=act_dummy[:].broadcast_to((p, wt)),
                    in_=yt[:, :wt],
                    func=mybir.ActivationFunctionType.Square,
                    accum_out=s1p[:, t : t + 1],
                )
            else:
                nc.vector.scalar_tensor_tensor(
                    out=dve_scr[:, :wt],
                    in0=yt[:, :wt],
                    scalar=1.0,
                    in1=yt[:, :wt],
                    op0=mybir.AluOpType.mult,
                    op1=mybir.AluOpType.mult,
                    accum_out=s1p[:, t : t + 1],
                )
            # T partial: fold x pairwise at TT 2x rate, then 1x accum tail.
            # The fold outputs pack into dve_scr back-to-back; the tail
            # writes the remaining scr space (exactly fills tile_w).
            src, off, cw = xt, 0, wt
            if wt == tile_w:
                for k in range(T_FOLDS):
                    half = cw // 2
                    dst = 0 if k == 0 else off + cw
                    nc.vector.tensor_tensor(
                        out=dve_scr[:, dst : dst + half],
                        in0=src[:, off : off + half],
                        in1=src[:, off + half : off + cw],
                        op=mybir.AluOpType.add,
                    )
                    src, off, cw = dve_scr, dst, half
            tail_out = (
                dve_scr[:, off + cw : off + 2 * cw]
                if src is dve_scr
                else dve_scr[:, :cw]
            )
            nc.vector.tensor_scalar(
                out=tail_out,
                in0=src[:, off : off + cw],
                scalar1=1.0,
                scalar2=0.0,
                op0=mybir.AluOpType.mult,
                op1=mybir.AluOpType.add,
                accum_out=txp[:, t : t + 1],
            )

        nc.vector.reduce_sum(
            out=stats_sb[:, 0:1], in_=s1p[:], axis=mybir.AxisListType.X
        )
        nc.vector.reduce_sum(
            out=stats_sb[:, 1:2], in_=s2p[:], axis=mybir.AxisListType.X
        )
        nc.vector.reduce_sum(
            out=stats_sb[:, 2:3], in_=txp[:], axis=mybir.AxisListType.X
        )
        nc.sync.dma_start(out=stats[:], in_=stats_sb[:])
    nc.compile()
    return nc


@functools.lru_cache(maxsize=1)
def _get_nc():
    return build_kernel()


def host_combine(stats, labels_flat, mask_flat, p_row):
    """Combine per-row device sums into the final scalar loss (float64)."""
    S1 = stats[:, 0].astype(np.float64)
    S2 = stats[:, 1].astype(np.float64)
    T = stats[:, 2].astype(np.float64)
    g = stats[:, 3].astype(np.float64)
    lse1 = np.log(S1)  # logsumexp(x) per row (x ~ N(0,1): no overflow)
    lse2 = np.log(S2)  # logsumexp(x/2) per row
    valid = labels_flat != -100
    n_valid = max(int(valid.sum()), 1)
    ce = float(np.sum(np.where(valid, lse1 - g, 0.0)) / n_valid)

    slp_sum = 0.5 * T - V * lse2  # sum_v log_softmax(x/2) per row
    logp = np.log(p_row)
    kl_token = V * p_row * logp - p_row * slp_sum
    kl_sum = float(np.sum(mask_flat * kl_token))
    msum = float(mask_flat.sum())
    kl = (kl_sum / msum if msum > 0 else kl_sum) * (TEMP**2)
    return CE_W * ce + KL_W * kl


def kernel(student_logits, teacher_token_logprobs, labels, attention_mask):
    x2d = (
        np.asarray(student_logits, dtype=np.float32)
        .reshape(B * S, V)
        .astype(ml_dtypes.bfloat16)
    )
    labels_flat = np.asarray(labels).reshape(-1).astype(np.int64)
    mask_flat = np.asarray(attention_mask).reshape(-1).astype(np.float64)
    tlp = np.asarray(teacher_token_logprobs, dtype=np.float64)
    prob = np.minimum(np.exp(tlp), 0.99)
    p_t = (1.0 - prob) / V  # [S]
    p_row = np.tile(p_t, B)  # [B*S] row-major (b, t)
    safe_labels = np.where(labels_flat < 0, 0, labels_flat)

    nc = _get_nc()
    in_maps = []
    for c in range(N_CORES):
        sl = slice(c * P, (c + 1) * P)
        flat = np.arange(P, dtype=np.int64) * V + safe_labels[sl]
        g_idx = (flat // GATHER_BLK).astype(np.int32)
        onehot = np.zeros((P, GATHER_BLK), dtype=ml_dtypes.bfloat16)
        onehot[np.arange(P), flat % GATHER_BLK] = 1.0
        in_maps.append(
            {"x": np.ascontiguousarray(x2d[sl]), "gidx": g_idx[:, None], "onehot": onehot}
        )
    trace = bool(int(os.environ.get("KERNEL_TRACE", "0")))
    res = run_bass_kernel_spmd(
        nc, in_maps, core_ids=list(range(N_CORES)), trace=trace
    )
    global _LAST_RESULTS
    _LAST_RESULTS = res
    stats = np.concatenate([r["stats"] for r in res.results], axis=0)
    total = host_combine(stats, labels_flat, mask_flat, p_row)
    return np.float32(total)


_LAST_RESULTS = None
